# revision 23
# baseline (speedup 1.0000x reference)
"""Trainium2 Bass kernel for nn_LLPKTMultiType (LLPKT knowledge tracing).

Strategy: pure data parallel, 4 samples per core x 8 cores.

The 850-step sequential erase-add memory recurrence
    M_{s+1}[c,d] = M_s[c,d] * (1 - w_s[c] e_s[d]) + w_s[c] a_s[d]
is evaluated with the DVE TensorTensorScan instruction (state = d0*state + d1
along the free axis).  Layout: partitions = (2 samples x 64 d), free =
(50 concepts x step positions), chunked over steps with a zero-multiplier
reset column per concept segment carrying the state across chunks.
Readouts sum_c w[c] M_s[c,d] use scalar_tensor_tensor with accum_out.

Program order is software-pipelined: pair-0 gathers, then pair-0 scan
chunks interleaved with pair-1 gathers, then pair-1 scan chunks
interleaved with pair-0 readout/MLP, then pair-1 readout/MLP.
"""

import os
import sys

import numpy as np

sys.path.insert(0, "/opt/trn_rl_repo")

B, S, L = 32, 50, 16
C, D = 50, 64
NQ, NL, NU = 10000, 2000, 5000
QV = NQ + NL + 1          # 12001
QAV = 2 * NQ + 1          # 20001
H4 = 4 * D                # 256
EPS = 1e-5

BL = 4                    # samples per core
NCORES = 8
NS = S * (L + 1)          # 850 flat update steps per sample
TC = 5                    # outer timesteps per scan chunk
SC = S // TC              # 10 chunks
CH = 17 * TC              # 85 update positions per chunk
LROWS = S * L             # 800 real lecture rows per sample
LPAD = 896                # padded to 7*128
QPAD = 128

_BUILT = None


def _build():
    import concourse.bass as bass
    import concourse.bacc as bacc
    import concourse.mybir as mybir
    import concourse.tile as tile
    from concourse.masks import make_identity

    f32 = mybir.dt.float32
    f16 = mybir.dt.float16
    i32 = mybir.dt.int32
    AX = mybir.AxisListType
    OP = mybir.AluOpType
    AF = mybir.ActivationFunctionType

    nc = bacc.Bacc("TRN2", target_bir_lowering=False, debug=False,
                   num_devices=NCORES)

    din = lambda n, sh, dt=f32: nc.dram_tensor(n, sh, dt, kind="ExternalInput").ap()
    ql_idx = din("ql_idx", [BL * LPAD], i32)
    q_idx = din("q_idx", [BL * QPAD], i32)
    qa_idx = din("qa_idx", [BL * QPAD], i32)
    le_idx = din("le_idx", [BL * QPAD], i32)
    q_embed = din("q_embed", [QV, D])
    qa_embed = din("qa_embed", [QAV, D])
    key = din("key", [C, D])
    M0 = din("M0", [C, D])
    W_ea = din("W_ea", [D + 1, 2 * D])           # W_e | W_a packed, + bias row
    W0 = din("W0", [H4, H4])
    W1 = din("W1", [H4, H4])
    Wout = din("Wout", [H4])
    biases = din("biases", [2 * H4 + 2 * D])     # b0 | b1 | b_e | b_a
    gamma_beta = din("gamma_beta", [2 * H4])
    b_out = din("b_out", [1])
    preds = nc.dram_tensor("preds", [BL, S], f32, kind="ExternalOutput").ap()

    with tile.TileContext(nc) as tc:
        with (
            tc.tile_pool(name="persist", bufs=1) as pp,
            tc.tile_pool(name="gather", bufs=4) as gp,
            tc.tile_pool(name="chunk2", bufs=2) as cp2,
            tc.tile_pool(name="chunk1", bufs=1) as cp1,
            tc.tile_pool(name="psum", bufs=2, space="PSUM") as psp,
            tc.tile_pool(name="dram", bufs=1, space="DRAM") as dp,
        ):
            # ---------------- constants / weights ----------------
            ident = pp.tile([128, 128], f32, tag="ident")
            make_identity(nc, ident[:])

            KT = pp.tile([D, C], f32, tag="KT")                    # [d, c]
            nc.sync.dma_start(KT[:], key.rearrange("c d -> d c"))
            Wea_sb = pp.tile([D + 1, 2 * D], f32, tag="Wea")
            nc.sync.dma_start(Wea_sb[:], W_ea)
            W01 = pp.tile([128, 4, H4], f32, tag="W01")            # W0lo W0hi W1lo W1hi
            nc.sync.dma_start(W01[:, 0, :], W0[0:128, :])
            nc.sync.dma_start(W01[:, 1, :], W0[128:256, :])
            nc.sync.dma_start(W01[:, 2, :], W1[0:128, :])
            nc.sync.dma_start(W01[:, 3, :], W1[128:256, :])
            Wout_rep = pp.tile([128, H4], f32, tag="Woutr")
            nc.sync.dma_start(Wout_rep[:], Wout[None, :].to_broadcast([128, H4]))
            bias_rep = pp.tile([128, 2 * H4 + 2 * D], f32, tag="biasr")
            nc.sync.dma_start(bias_rep[:],
                              biases[None, :].to_broadcast([128, 2 * H4 + 2 * D]))
            gb_rep = pp.tile([S, 2 * H4], f32, tag="gbr")
            nc.sync.dma_start(gb_rep[:], gamma_beta[None, :].to_broadcast([S, 2 * H4]))
            bout_rep = pp.tile([128, 1], f32, tag="boutr")
            nc.sync.dma_start(bout_rep[:], b_out[None, :].to_broadcast([128, 1]))
            # M0 transposed + replicated: partitions (2b x 64d), free c
            M0T = pp.tile([128, C], f32, tag="M0T")
            for bb in range(2):
                nc.sync.dma_start(M0T[D * bb:D * bb + D, :],
                                  M0.rearrange("c d -> d c"))
            ones50 = pp.tile([128, C], f32, tag="ones50")
            nc.gpsimd.memset(ones50[:], 1.0)

            w_dram = dp.tile([BL, NS, C], f32, tag="wdram")

            E = [pp.tile([128, NS], f32, tag=f"E{p}", name=f"E{p}") for p in range(2)]
            A = [pp.tile([128, NS], f32, tag=f"A{p}", name=f"A{p}") for p in range(2)]
            lr = [pp.tile([128, S], f32, tag=f"lr{p}", name=f"lr{p}") for p in range(2)]
            qr = [pp.tile([128, S], f32, tag=f"qr{p}", name=f"qr{p}") for p in range(2)]
            q_raw = [pp.tile([128, D], f32, tag=f"qraw{b}", name=f"qraw{b}") for b in range(BL)]
            le_raw = [pp.tile([128, D], f32, tag=f"leraw{b}", name=f"leraw{b}") for b in range(BL)]

            # ---------------- gather + dense phase ----------------
            def gather_chunk(idx_dram, off, table, dst=None):
                it = gp.tile([128, 1], i32, tag="idx")
                nc.sync.dma_start(it[:], idx_dram[off:off + 128][:, None])
                g = dst if dst is not None else gp.tile([128, D], f32, tag="graw", name="graw")
                nc.gpsimd.indirect_dma_start(
                    out=g[:], out_offset=None, in_=table,
                    in_offset=bass.IndirectOffsetOnAxis(ap=it[:, :1], axis=0))
                return g

            def xT_of(g, tag):
                """Transpose gathered rows to [d, rows]; append a ones row so
                the gates matmul picks up its bias from Wea's bias row."""
                ps = psp.tile([D, 128], f32, space="PSUM", tag="tp")
                nc.tensor.transpose(out=ps[:], in_=g[:], identity=ident[:])
                xT = gp.tile([D + 1, 128], f32, tag=tag, name=tag)
                nc.scalar.activation(xT[0:D, :], ps[:], AF.Copy)
                nc.gpsimd.memset(xT[D:D + 1, :], 1.0)
                return xT

            def gates_psum(xT):
                """[128 rows, er|ad] -> transposed PSUM tile.  Bias comes from
                the ones row; tanh(x) = 2*sigmoid(2x)-1 (fixed at drain) so
                only the Sigmoid table is ever loaded here."""
                psg = psp.tile([128, 2 * D], f32, space="PSUM", tag="gates")
                nc.tensor.matmul(psg[:], lhsT=xT[:], rhs=Wea_sb[:],
                                 start=True, stop=True)
                gs = gp.tile([128, 2 * D], f32, tag="gsig", name="gsig")
                nc.scalar.activation(gs[:, 0:D], psg[:, 0:D], AF.Sigmoid)
                nc.scalar.activation(gs[:, D:2 * D], psg[:, D:2 * D], AF.Sigmoid,
                                     scale=2.0)
                pst = psp.tile([128, 128], f32, space="PSUM", tag="tp")
                nc.tensor.transpose(out=pst[:], in_=gs[:], identity=ident[:])
                return pst

            def corr_w(xT):
                """softmax(x @ K^T) over c -> [128, C] sbuf tile.  Logits are
                tiny (embeddings ~0.02 scale) so the max-shift is skipped;
                normalization runs on the Scalar engine (scale=1/sum)."""
                psc = psp.tile([128, C], f32, space="PSUM", tag="corr")
                nc.tensor.matmul(psc[:], lhsT=xT[0:D, :], rhs=KT[:],
                                 start=True, stop=True)
                wexp = gp.tile([128, C], f32, tag="wexp")
                se = gp.tile([128, 1], f32, tag="se")
                nc.scalar.activation(wexp[:], psc[:], AF.Exp,
                                     accum_out=se[:, :1])
                rse = gp.tile([128, 1], f32, tag="rse")
                nc.vector.reciprocal(rse[:], se[:])
                wsb = gp.tile([128, C], f32, tag="wsb")
                nc.scalar.activation(wsb[:], wexp[:], AF.Copy,
                                     scale=rse[:, :1])
                return wsb

            def gather_sample(b):
                """All gathers + transposes for sample b; returns xT tiles."""
                xT_l = []
                for j in range(7):
                    g = gather_chunk(ql_idx, b * LPAD + 128 * j, q_embed)
                    xT_l.append(xT_of(g, f"xT{j}"))
                gq = gather_chunk(q_idx, b * QPAD, q_embed, dst=q_raw[b])
                xT_q = xT_of(gq, "xTq")
                gqa = gather_chunk(qa_idx, b * QPAD, qa_embed)
                xT_qa = xT_of(gqa, "xTqa")
                gather_chunk(le_idx, b * QPAD, q_embed, dst=le_raw[b])
                return xT_l, xT_q, xT_qa

            def corr_sample(b, xT_l, xT_q):
                """All softmax correlations for sample b (Exp table)."""
                wd3 = w_dram[b, :, :].rearrange("(t k) c -> t k c", k=17)
                for j in range(7):
                    t0 = 8 * j
                    tcnt = min(8, S - t0)
                    wsb = corr_w(xT_l[j])
                    nc.sync.dma_start(wd3[t0:t0 + tcnt, 0:16, :],
                                      wsb[0:tcnt * 16, :])
                wsb = corr_w(xT_q)
                nc.sync.dma_start(wd3[0:S, 16, :], wsb[0:S, :])

            def gates_sample(b, xT_l, xT_qa):
                """All erase/add gates for sample b (Sigmoid table)."""
                pr, half = b // 2, 64 * (b % 2)
                E3 = E[pr][half:half + D, :].rearrange("p (t k) -> p t k", k=17)
                A3 = A[pr][half:half + D, :].rearrange("p (t k) -> p t k", k=17)
                for j in range(7):
                    t0 = 8 * j
                    tcnt = min(8, S - t0)
                    pst = gates_psum(xT_l[j])
                    pst3 = pst[:].rearrange("p (t k) -> p t k", k=16)
                    nc.scalar.activation(E3[:, t0:t0 + tcnt, 0:16],
                                         pst3[0:D, 0:tcnt, :], AF.Copy)
                    nc.scalar.activation(A3[:, t0:t0 + tcnt, 0:16],
                                         pst3[D:2 * D, 0:tcnt, :], AF.Copy,
                                         scale=2.0, bias=-1.0)
                pst = gates_psum(xT_qa)
                nc.scalar.activation(E3[:, 0:S, 16], pst[0:D, 0:S], AF.Copy)
                nc.scalar.activation(A3[:, 0:S, 16], pst[D:2 * D, 0:S], AF.Copy,
                                     scale=2.0, bias=-1.0)

            def gather_pair(pr):
                """Emission pieces (thunks) for pair pr, ordered to minimize
                activation-table reloads: gathers, then all Exp work, then
                all Sigmoid work."""
                state = {}

                def mk_gather(b):
                    return lambda: state.__setitem__(b, gather_sample(b))

                def mk_corr(b):
                    return lambda: corr_sample(b, state[b][0], state[b][1])

                def mk_gates(b):
                    return lambda: gates_sample(b, state[b][0], state[b][2])

                b0, b1 = 2 * pr, 2 * pr + 1
                return [mk_gather(b0), mk_gather(b1),
                        mk_corr(b0), mk_corr(b1),
                        mk_gates(b0), mk_gates(b1)]

            # ---------------- scan phase ----------------
            Mts = {}
            B86 = CH + 1  # 86 columns per c-block (reset col + 85 updates)
            Sfull = [pp.tile([128, SC * B86], f32, tag=f"Sf{p}", name=f"Sf{p}")
                     for p in range(2)]

            def scan_chunk(pr, ch):
                s0 = CH * ch
                # wb stored s-major (contiguous replicate-DMA); compute
                # ops read it through a permuted (c-outer) view
                wb = cp2.tile([128, C * CH], f32, tag="wb")
                wb3 = wb[:].rearrange("p (s c) -> p s c", c=C) \
                           .rearrange("p s c -> p c s")
                for bb in range(2):
                    srcb = w_dram[2 * pr + bb, s0:s0 + CH, :]
                    srcb = srcb[None, :, :].to_broadcast([D, CH, C])
                    nc.sync.dma_start(wb[D * bb:D * bb + D, 0:2000],
                                      srcb[:, 0:40, :])
                    nc.sync.dma_start(wb[D * bb:D * bb + D, 2000:C * CH],
                                      srcb[:, 40:CH, :])

                u = cp2.tile([128, C * (CH + 1)], f16, tag="u")
                v = cp2.tile([128, C * (CH + 1)], f16, tag="v")
                Mt = cp1.tile([128, C * (CH + 1)], f32, tag="Mt")
                u3 = u[:].rearrange("p (c s) -> p c s", s=CH + 1)
                v3 = v[:].rearrange("p (c s) -> p c s", s=CH + 1)
                Mt3 = Mt[:].rearrange("p (c s) -> p c s", s=CH + 1)

                Eb = E[pr][:, s0:s0 + CH][:, None, :].to_broadcast([128, C, CH])
                Ab = A[pr][:, s0:s0 + CH][:, None, :].to_broadcast([128, C, CH])
                nc.vector.tensor_tensor(u3[:, :, 1:], wb3, Eb, op=OP.mult)
                nc.scalar.activation(u3[:, :, 1:], u3[:, :, 1:], AF.Copy,
                                     bias=1.0, scale=-1.0)
                nc.gpsimd.memset(u3[:, :, 0:1], 0.0)
                nc.gpsimd.tensor_tensor(v3[:, :, 1:], wb3, Ab, op=OP.mult)
                # reset column: carry the state across chunks.  STT instead of
                # tensor_copy: strided-src tensor_copy hits a pathological DVE
                # mode (~11us for 50 elements).
                if ch == 0:
                    carry_src = M0T[:][:, :, None]
                else:
                    carry_src = Mts[pr][:, :, CH:CH + 1]
                nc.vector.scalar_tensor_tensor(
                    out=v3[:, :, 0:1], in0=carry_src, scalar=1.0,
                    op0=OP.mult, in1=ones50[:][:, :, None], op1=OP.mult)

                nc.vector.tensor_tensor_scan(
                    Mt[:], u[:], v[:], 0.0, op0=OP.mult, op1=OP.add)
                Mts[pr] = Mt3

                # q_read: small per-step STTs against the step-start state
                scr2 = cp2.tile([128, C], f32, tag="scr2")
                for tl in range(TC):
                    t = TC * ch + tl
                    sl = 17 * tl
                    nc.vector.scalar_tensor_tensor(
                        out=scr2[:][:, :, None],
                        in0=wb3[:, :, sl + 16:sl + 17],
                        scalar=1.0, op0=OP.mult, in1=Mt3[:, :, sl:sl + 1],
                        op1=OP.mult, accum_out=qr[pr][:, t:t + 1])

                # S-tree: SS[p, s] = sum_c Mt[p, c, s] (for telescoped l_read)
                # 50 = 25+25 -> 12+12 (+1 leftover) -> 6 -> 3 -> 1+1+1
                t25 = cp2.tile([128, 25 * B86], f32, tag="t25")
                nc.gpsimd.tensor_tensor(t25[:], Mt[:, 0:25 * B86],
                                        Mt[:, 25 * B86:50 * B86], op=OP.add)
                t12 = cp1.tile([128, 12 * B86], f32, tag="t12")
                nc.vector.tensor_tensor(t12[:], t25[:, 0:12 * B86],
                                        t25[:, 12 * B86:24 * B86], op=OP.add)
                t6 = cp1.tile([128, 6 * B86], f32, tag="t6")
                nc.vector.tensor_tensor(t6[:], t12[:, 0:6 * B86],
                                        t12[:, 6 * B86:12 * B86], op=OP.add)
                t3 = cp1.tile([128, 3 * B86], f32, tag="t3")
                nc.vector.tensor_tensor(t3[:], t6[:, 0:3 * B86],
                                        t6[:, 3 * B86:6 * B86], op=OP.add)
                t1 = cp1.tile([128, B86], f32, tag="t1")
                nc.vector.tensor_tensor(t1[:], t3[:, 0:B86],
                                        t3[:, B86:2 * B86], op=OP.add)
                nc.vector.tensor_tensor(t1[:], t1[:],
                                        t3[:, 2 * B86:3 * B86], op=OP.add)
                nc.vector.tensor_tensor(
                    Sfull[pr][:, ch * B86:(ch + 1) * B86], t1[:],
                    t25[:, 24 * B86:25 * B86], op=OP.add)

            def fixup_pair(pr):
                """Telescoped l_read: for each write, read = (S_before -
                S_after + a) / e; sum the 16 lecture reads per step."""
                Sf3 = Sfull[pr][:].rearrange("p (h s) -> p h s", s=B86)
                ds = cp1.tile([128, NS], f32, tag="ds")
                ds3 = ds[:].rearrange("p (h s) -> p h s", s=CH)
                nc.vector.tensor_tensor(ds3, Sf3[:, :, 0:CH],
                                        Sf3[:, :, 1:B86], op=OP.subtract)
                # r = 1/E by two Newton steps from r0=2 (E = sigmoid, ~0.5)
                r = cp1.tile([128, NS], f32, tag="rr")
                tn = cp1.tile([128, NS], f32, tag="tn")
                nc.scalar.activation(r[:], E[pr][:, :], AF.Copy,
                                     scale=-4.0, bias=4.0)
                nc.vector.tensor_tensor(tn[:], E[pr][:, :], r[:], op=OP.mult)
                nc.scalar.activation(tn[:], tn[:], AF.Copy,
                                     scale=-1.0, bias=2.0)
                nc.vector.tensor_tensor(r[:], r[:], tn[:], op=OP.mult)
                nc.vector.tensor_tensor(tn[:], E[pr][:, :], r[:], op=OP.mult)
                nc.scalar.activation(tn[:], tn[:], AF.Copy,
                                     scale=-1.0, bias=2.0)
                nc.vector.tensor_tensor(r[:], r[:], tn[:], op=OP.mult)
                # zero the qa positions so the 17-wide reduce skips them
                r3 = r[:].rearrange("p (t k) -> p t k", k=17)
                nc.gpsimd.memset(r3[:, :, 16:17], 0.0)
                nc.vector.tensor_tensor(ds[:], ds[:], A[pr][:, :], op=OP.add)
                nc.vector.tensor_tensor(ds[:], ds[:], r[:], op=OP.mult)
                nc.vector.tensor_reduce(
                    lr[pr][:, :], ds[:].rearrange("p (t k) -> p t k", k=17),
                    axis=AX.X, op=OP.add)

            # ---------------- readout: mastery -> LN -> MLP ----------------
            msT_lo = pp.tile([128, BL * S], f32, tag="msTlo")
            msT_hi = pp.tile([128, BL * S], f32, tag="msThi")

            def readout_pair(pr):
                ms = pp.tile([S, 2 * H4], f32, tag=f"ms{pr}")
                for which, tsrc in ((0, qr[pr]), (2, lr[pr])):
                    pst = psp.tile([S, 128], f32, space="PSUM", tag="tp")
                    nc.tensor.transpose(out=pst[:], in_=tsrc[:], identity=ident[:])
                    for bh in range(2):
                        nc.vector.tensor_copy(
                            ms[:, bh * H4 + which * D:bh * H4 + (which + 1) * D],
                            pst[:, bh * D:(bh + 1) * D])
                for bh in range(2):
                    b = 2 * pr + bh
                    nc.vector.tensor_copy(ms[:, bh * H4 + D:bh * H4 + 2 * D],
                                          q_raw[b][0:S, :])
                    nc.vector.tensor_copy(ms[:, bh * H4 + 3 * D:bh * H4 + 4 * D],
                                          le_raw[b][0:S, :])
                ms3 = ms[:].rearrange("p (b f) -> p b f", f=H4)
                mean = pp.tile([S, 2], f32, tag=f"mean{pr}")
                nc.vector.tensor_reduce(mean[:], ms3, axis=AX.X, op=OP.add)
                nc.vector.tensor_scalar_mul(mean[:], mean[:], 1.0 / H4)
                mb = mean[:][:, :, None].to_broadcast([S, 2, H4])
                nc.vector.tensor_tensor(ms3, ms3, mb, op=OP.subtract)
                sq = pp.tile([S, 2 * H4], f32, tag=f"sq{pr}")
                nc.scalar.activation(sq[:], ms[:], AF.Square)
                var = pp.tile([S, 2], f32, tag=f"var{pr}")
                nc.vector.tensor_reduce(
                    var[:], sq[:].rearrange("p (b f) -> p b f", f=H4),
                    axis=AX.X, op=OP.add)
                nc.vector.tensor_scalar(var[:], var[:], 1.0 / H4, EPS,
                                        op0=OP.mult, op1=OP.add)
                sd = pp.tile([S, 2], f32, tag=f"sd{pr}")
                nc.scalar.activation(sd[:], var[:], AF.Sqrt)
                rsd = pp.tile([S, 2], f32, tag=f"rsd{pr}")
                nc.vector.reciprocal(rsd[:], sd[:])
                nc.vector.tensor_tensor(
                    ms3, ms3, rsd[:][:, :, None].to_broadcast([S, 2, H4]),
                    op=OP.mult)
                gmb = gb_rep[:, 0:H4][:, None, :].to_broadcast([S, 2, H4])
                btb = gb_rep[:, H4:2 * H4][:, None, :].to_broadcast([S, 2, H4])
                nc.vector.tensor_tensor(ms3, ms3, gmb, op=OP.mult)
                nc.vector.tensor_tensor(ms3, ms3, btb, op=OP.add)
                for bh in range(2):
                    b = 2 * pr + bh
                    for fh, dstT in ((0, msT_lo), (1, msT_hi)):
                        pst = psp.tile([128, S], f32, space="PSUM", tag="tp")
                        nc.tensor.transpose(
                            out=pst[:],
                            in_=ms[:, bh * H4 + fh * 128:bh * H4 + (fh + 1) * 128],
                            identity=ident[0:S, 0:S])
                        nc.vector.tensor_copy(dstT[:, b * S:(b + 1) * S], pst[:])

            def mlp_pair(rc):
                rows = 2 * S  # 100 rows: (b within pair, t)
                csl = slice(rc * rows, (rc + 1) * rows)
                ph = psp.tile([rows, H4], f32, space="PSUM", tag="mlp")
                nc.tensor.matmul(ph[:], lhsT=msT_lo[:, csl], rhs=W01[:, 0, :],
                                 start=True, stop=False)
                nc.tensor.matmul(ph[:], lhsT=msT_hi[:, csl], rhs=W01[:, 1, :],
                                 start=False, stop=True)
                h1 = pp.tile([rows, H4], f32, tag=f"h1_{rc}")
                nc.vector.tensor_tensor(h1[:], ph[:], bias_rep[0:rows, 0:H4],
                                        op=OP.add)
                nc.scalar.activation(h1[:], h1[:], AF.Relu)
                h1T = [pp.tile([128, rows], f32, tag=f"h1T{fh}_{rc}", name=f"h1T{fh}_{rc}")
                       for fh in range(2)]
                for fh in range(2):
                    pst = psp.tile([128, rows], f32, space="PSUM", tag="tp")
                    nc.tensor.transpose(out=pst[:],
                                        in_=h1[:, fh * 128:(fh + 1) * 128],
                                        identity=ident[0:rows, 0:rows])
                    nc.vector.tensor_copy(h1T[fh][:], pst[:])
                ph2 = psp.tile([rows, H4], f32, space="PSUM", tag="mlp")
                nc.tensor.matmul(ph2[:], lhsT=h1T[0][:], rhs=W01[:, 2, :],
                                 start=True, stop=False)
                nc.tensor.matmul(ph2[:], lhsT=h1T[1][:], rhs=W01[:, 3, :],
                                 start=False, stop=True)
                h2 = pp.tile([rows, H4], f32, tag=f"h2_{rc}")
                nc.vector.tensor_tensor(h2[:], ph2[:],
                                        bias_rep[0:rows, H4:2 * H4], op=OP.add)
                scr4 = pp.tile([rows, H4], f32, tag=f"scr4_{rc}")
                logit = pp.tile([rows, 1], f32, tag=f"logit{rc}")
                nc.vector.scalar_tensor_tensor(
                    out=scr4[:], in0=h2[:], scalar=1.0, op0=OP.mult,
                    in1=Wout_rep[0:rows, :], op1=OP.mult,
                    accum_out=logit[:, 0:1])
                psig = pp.tile([rows, 1], f32, tag=f"psig{rc}")
                nc.scalar.activation(psig[:], logit[:], AF.Sigmoid,
                                     bias=bout_rep[0:rows, 0:1], scale=1.0)
                nc.sync.dma_start(
                    preds[2 * rc:2 * rc + 2, :].rearrange("b t -> (b t)")[:, None],
                    psig[:, 0:1])

            # ---------------- pipelined emission ----------------
            # pair-0 gather
            for piece in gather_pair(0):
                piece()
            # pair-0 scan, pair-1 gather interleaved
            p1_pieces = gather_pair(1)
            for ch in range(SC):
                scan_chunk(0, ch)
                if ch < len(p1_pieces):
                    p1_pieces[ch]()
            fixup_pair(0)
            # pair-1 scan, pair-0 readout/MLP interleaved
            for ch in range(SC):
                scan_chunk(1, ch)
                if ch == 0:
                    readout_pair(0)
                elif ch == 1:
                    mlp_pair(0)
            fixup_pair(1)
            readout_pair(1)
            mlp_pair(1)

    nc.compile()
    return nc


def _host_prepare(inputs):
    q_data = np.asarray(inputs["q_data"]).astype(np.int32)
    qa_data = np.asarray(inputs["qa_data"]).astype(np.int32)
    l_data = np.asarray(inputs["l_data"]).astype(np.int32)
    f = lambda k: np.ascontiguousarray(np.asarray(inputs[k]), dtype=np.float32)
    q_embed, qa_embed = f("q_embed"), f("qa_embed")
    key, M0 = f("key_matrix"), f("M0")
    W_ea = np.concatenate([f("W_e"), f("W_a")], axis=1)
    W_ea = np.concatenate(
        [W_ea, np.concatenate([f("b_e"), f("b_a")])[None, :]], axis=0)
    biases = np.concatenate([f("b0"), f("b1"), f("b_e"), f("b_a")])
    gamma_beta = np.concatenate([f("ln_gamma"), f("ln_beta")])
    W0, W1 = f("W0"), f("W1")
    Wout = f("W_out").reshape(-1)
    b_out = f("b_out").reshape(-1)

    in_maps = []
    for core in range(NCORES):
        bs = slice(core * BL, (core + 1) * BL)
        ql = np.zeros((BL, LPAD), np.int32)
        ql[:, :LROWS] = l_data[bs].reshape(BL, LROWS)
        qi = np.zeros((BL, QPAD), np.int32)
        qi[:, :S] = q_data[bs]
        qai = np.zeros((BL, QPAD), np.int32)
        qai[:, :S] = qa_data[bs]
        lei = np.zeros((BL, QPAD), np.int32)
        lei[:, :S] = l_data[bs][:, :, L - 1]
        in_maps.append(dict(
            ql_idx=np.ascontiguousarray(ql.reshape(-1)),
            q_idx=np.ascontiguousarray(qi.reshape(-1)),
            qa_idx=np.ascontiguousarray(qai.reshape(-1)),
            le_idx=np.ascontiguousarray(lei.reshape(-1)),
            q_embed=q_embed, qa_embed=qa_embed, key=key, M0=M0,
            W_ea=W_ea, W0=W0, W1=W1, Wout=Wout, biases=biases,
            gamma_beta=gamma_beta, b_out=b_out,
        ))
    return in_maps


def kernel(**inputs):
    global _BUILT
    if _BUILT is None:
        _BUILT = _build()
    nc = _BUILT
    from concourse import bass_utils
    in_maps = _host_prepare(inputs)
    res = bass_utils.run_bass_kernel_spmd(
        nc, in_maps, core_ids=list(range(NCORES)),
        trace=bool(int(os.environ.get("KERNEL_TRACE", "0"))))
    out = np.concatenate([r["preds"] for r in res.results], axis=0)
    kernel.last_results = res
    return out


# revision 24
# speedup vs baseline: 1.0000x; 1.0000x over previous
"""Trainium2 Bass kernel for nn_LLPKTMultiType (LLPKT knowledge tracing).

Strategy: pure data parallel, 4 samples per core x 8 cores.

The 850-step sequential erase-add memory recurrence
    M_{s+1}[c,d] = M_s[c,d] * (1 - w_s[c] e_s[d]) + w_s[c] a_s[d]
is evaluated with the DVE TensorTensorScan instruction (state = d0*state + d1
along the free axis).  Layout: partitions = (2 samples x 64 d), free =
(50 concepts x step positions), chunked over steps with a zero-multiplier
reset column per concept segment carrying the state across chunks.
Readouts sum_c w[c] M_s[c,d] use scalar_tensor_tensor with accum_out.

Program order is software-pipelined: pair-0 gathers, then pair-0 scan
chunks interleaved with pair-1 gathers, then pair-1 scan chunks
interleaved with pair-0 readout/MLP, then pair-1 readout/MLP.
"""

import os
import sys

import numpy as np

sys.path.insert(0, "/opt/trn_rl_repo")

B, S, L = 32, 50, 16
C, D = 50, 64
NQ, NL, NU = 10000, 2000, 5000
QV = NQ + NL + 1          # 12001
QAV = 2 * NQ + 1          # 20001
H4 = 4 * D                # 256
EPS = 1e-5

BL = 4                    # samples per core
NCORES = 8
NS = S * (L + 1)          # 850 flat update steps per sample
TC = 5                    # outer timesteps per scan chunk
SC = S // TC              # 10 chunks
CH = 17 * TC              # 85 update positions per chunk
LROWS = S * L             # 800 real lecture rows per sample
LPAD = 896                # padded to 7*128
QPAD = 128

_BUILT = None


def _build():
    import concourse.bass as bass
    import concourse.bacc as bacc
    import concourse.mybir as mybir
    import concourse.tile as tile
    from concourse.masks import make_identity

    f32 = mybir.dt.float32
    f16 = mybir.dt.float16
    i32 = mybir.dt.int32
    AX = mybir.AxisListType
    OP = mybir.AluOpType
    AF = mybir.ActivationFunctionType

    nc = bacc.Bacc("TRN2", target_bir_lowering=False, debug=False,
                   num_devices=NCORES)

    din = lambda n, sh, dt=f32: nc.dram_tensor(n, sh, dt, kind="ExternalInput").ap()
    ql_idx = din("ql_idx", [BL * LPAD], i32)
    q_idx = din("q_idx", [BL * QPAD], i32)
    qa_idx = din("qa_idx", [BL * QPAD], i32)
    le_idx = din("le_idx", [BL * QPAD], i32)
    q_embed = din("q_embed", [QV, D])
    qa_embed = din("qa_embed", [QAV, D])
    key = din("key", [C, D])
    M0 = din("M0", [C, D])
    W_ea = din("W_ea", [D + 1, 2 * D])           # W_e | W_a packed, + bias row
    W0 = din("W0", [H4, H4])
    W1 = din("W1", [H4, H4])
    Wout = din("Wout", [H4])
    biases = din("biases", [2 * H4 + 2 * D])     # b0 | b1 | b_e | b_a
    gamma_beta = din("gamma_beta", [2 * H4])
    b_out = din("b_out", [1])
    preds = nc.dram_tensor("preds", [BL, S], f32, kind="ExternalOutput").ap()

    with tile.TileContext(nc) as tc:
        with (
            tc.tile_pool(name="persist", bufs=1) as pp,
            tc.tile_pool(name="gather", bufs=4) as gp,
            tc.tile_pool(name="chunk2", bufs=2) as cp2,
            tc.tile_pool(name="chunk1", bufs=1) as cp1,
            tc.tile_pool(name="psum", bufs=2, space="PSUM") as psp,
            tc.tile_pool(name="dram", bufs=1, space="DRAM") as dp,
        ):
            # ---------------- constants / weights ----------------
            ident = pp.tile([128, 128], f32, tag="ident")
            make_identity(nc, ident[:])

            KT = pp.tile([D, C], f32, tag="KT")                    # [d, c]
            nc.sync.dma_start(KT[:], key.rearrange("c d -> d c"))
            Wea_sb = pp.tile([D + 1, 2 * D], f32, tag="Wea")
            nc.sync.dma_start(Wea_sb[:], W_ea)
            W01 = pp.tile([128, 4, H4], f32, tag="W01")            # W0lo W0hi W1lo W1hi
            nc.sync.dma_start(W01[:, 0, :], W0[0:128, :])
            nc.sync.dma_start(W01[:, 1, :], W0[128:256, :])
            nc.sync.dma_start(W01[:, 2, :], W1[0:128, :])
            nc.sync.dma_start(W01[:, 3, :], W1[128:256, :])
            Wout_rep = pp.tile([128, H4], f32, tag="Woutr")
            nc.sync.dma_start(Wout_rep[:], Wout[None, :].to_broadcast([128, H4]))
            bias_rep = pp.tile([128, 2 * H4 + 2 * D], f32, tag="biasr")
            nc.sync.dma_start(bias_rep[:],
                              biases[None, :].to_broadcast([128, 2 * H4 + 2 * D]))
            gb_rep = pp.tile([S, 2 * H4], f32, tag="gbr")
            nc.sync.dma_start(gb_rep[:], gamma_beta[None, :].to_broadcast([S, 2 * H4]))
            bout_rep = pp.tile([128, 1], f32, tag="boutr")
            nc.sync.dma_start(bout_rep[:], b_out[None, :].to_broadcast([128, 1]))
            # M0 transposed + replicated: partitions (2b x 64d), free c
            M0T = pp.tile([128, C], f32, tag="M0T")
            for bb in range(2):
                nc.sync.dma_start(M0T[D * bb:D * bb + D, :],
                                  M0.rearrange("c d -> d c"))
            ones50 = pp.tile([128, C], f32, tag="ones50")
            nc.gpsimd.memset(ones50[:], 1.0)

            w_dram = dp.tile([BL, NS, C], f32, tag="wdram")

            E = [pp.tile([128, NS], f32, tag=f"E{p}", name=f"E{p}") for p in range(2)]
            A = [pp.tile([128, NS], f32, tag=f"A{p}", name=f"A{p}") for p in range(2)]
            lr = [pp.tile([128, S], f32, tag=f"lr{p}", name=f"lr{p}") for p in range(2)]
            qr = [pp.tile([128, S], f32, tag=f"qr{p}", name=f"qr{p}") for p in range(2)]
            q_raw = [pp.tile([128, D], f32, tag=f"qraw{b}", name=f"qraw{b}") for b in range(BL)]
            le_raw = [pp.tile([128, D], f32, tag=f"leraw{b}", name=f"leraw{b}") for b in range(BL)]

            # ---------------- gather + dense phase ----------------
            def gather_chunk(idx_dram, off, table, dst=None):
                it = gp.tile([128, 1], i32, tag="idx")
                nc.sync.dma_start(it[:], idx_dram[off:off + 128][:, None])
                g = dst if dst is not None else gp.tile([128, D], f32, tag="graw", name="graw")
                nc.gpsimd.indirect_dma_start(
                    out=g[:], out_offset=None, in_=table,
                    in_offset=bass.IndirectOffsetOnAxis(ap=it[:, :1], axis=0))
                return g

            def xT_of(g, tag):
                """Transpose gathered rows to [d, rows]; append a ones row so
                the gates matmul picks up its bias from Wea's bias row."""
                ps = psp.tile([D, 128], f32, space="PSUM", tag="tp")
                nc.tensor.transpose(out=ps[:], in_=g[:], identity=ident[:])
                xT = gp.tile([D + 1, 128], f32, tag=tag, name=tag)
                nc.scalar.activation(xT[0:D, :], ps[:], AF.Copy)
                nc.gpsimd.memset(xT[D:D + 1, :], 1.0)
                return xT

            def gates_psum(xT):
                """[128 rows, er|ad] -> transposed PSUM tile.  Bias comes from
                the ones row; tanh(x) = 2*sigmoid(2x)-1 (fixed at drain) so
                only the Sigmoid table is ever loaded here."""
                psg = psp.tile([128, 2 * D], f32, space="PSUM", tag="gates")
                nc.tensor.matmul(psg[:], lhsT=xT[:], rhs=Wea_sb[:],
                                 start=True, stop=True)
                gs = gp.tile([128, 2 * D], f32, tag="gsig", name="gsig")
                nc.scalar.activation(gs[:, 0:D], psg[:, 0:D], AF.Sigmoid)
                nc.scalar.activation(gs[:, D:2 * D], psg[:, D:2 * D], AF.Sigmoid,
                                     scale=2.0)
                pst = psp.tile([128, 128], f32, space="PSUM", tag="tp")
                nc.tensor.transpose(out=pst[:], in_=gs[:], identity=ident[:])
                return pst

            def corr_w(xT):
                """softmax(x @ K^T) over c -> [128, C] sbuf tile.  Logits are
                tiny (embeddings ~0.02 scale) so the max-shift is skipped;
                normalization runs on the Scalar engine (scale=1/sum)."""
                psc = psp.tile([128, C], f32, space="PSUM", tag="corr")
                nc.tensor.matmul(psc[:], lhsT=xT[0:D, :], rhs=KT[:],
                                 start=True, stop=True)
                wexp = gp.tile([128, C], f32, tag="wexp")
                se = gp.tile([128, 1], f32, tag="se")
                nc.scalar.activation(wexp[:], psc[:], AF.Exp,
                                     accum_out=se[:, :1])
                rse = gp.tile([128, 1], f32, tag="rse")
                nc.vector.reciprocal(rse[:], se[:])
                wsb = gp.tile([128, C], f32, tag="wsb")
                nc.scalar.activation(wsb[:], wexp[:], AF.Copy,
                                     scale=rse[:, :1])
                return wsb

            def gather_sample(b):
                """All gathers + transposes for sample b; returns xT tiles."""
                xT_l = []
                for j in range(7):
                    g = gather_chunk(ql_idx, b * LPAD + 128 * j, q_embed)
                    xT_l.append(xT_of(g, f"xT{j}"))
                gq = gather_chunk(q_idx, b * QPAD, q_embed, dst=q_raw[b])
                xT_q = xT_of(gq, "xTq")
                gqa = gather_chunk(qa_idx, b * QPAD, qa_embed)
                xT_qa = xT_of(gqa, "xTqa")
                gather_chunk(le_idx, b * QPAD, q_embed, dst=le_raw[b])
                return xT_l, xT_q, xT_qa

            def corr_sample(b, xT_l, xT_q):
                """All softmax correlations for sample b (Exp table)."""
                wd3 = w_dram[b, :, :].rearrange("(t k) c -> t k c", k=17)
                for j in range(7):
                    t0 = 8 * j
                    tcnt = min(8, S - t0)
                    wsb = corr_w(xT_l[j])
                    nc.sync.dma_start(wd3[t0:t0 + tcnt, 0:16, :],
                                      wsb[0:tcnt * 16, :])
                wsb = corr_w(xT_q)
                nc.sync.dma_start(wd3[0:S, 16, :], wsb[0:S, :])

            def gates_sample(b, xT_l, xT_qa):
                """All erase/add gates for sample b (Sigmoid table)."""
                pr, half = b // 2, 64 * (b % 2)
                E3 = E[pr][half:half + D, :].rearrange("p (t k) -> p t k", k=17)
                A3 = A[pr][half:half + D, :].rearrange("p (t k) -> p t k", k=17)
                for j in range(7):
                    t0 = 8 * j
                    tcnt = min(8, S - t0)
                    pst = gates_psum(xT_l[j])
                    pst3 = pst[:].rearrange("p (t k) -> p t k", k=16)
                    nc.scalar.activation(E3[:, t0:t0 + tcnt, 0:16],
                                         pst3[0:D, 0:tcnt, :], AF.Copy)
                    nc.scalar.activation(A3[:, t0:t0 + tcnt, 0:16],
                                         pst3[D:2 * D, 0:tcnt, :], AF.Copy,
                                         scale=2.0, bias=-1.0)
                pst = gates_psum(xT_qa)
                nc.scalar.activation(E3[:, 0:S, 16], pst[0:D, 0:S], AF.Copy)
                nc.scalar.activation(A3[:, 0:S, 16], pst[D:2 * D, 0:S], AF.Copy,
                                     scale=2.0, bias=-1.0)

            def gather_pair(pr):
                """Emission pieces (thunks) for pair pr, ordered to minimize
                activation-table reloads: gathers, then all Exp work, then
                all Sigmoid work."""
                state = {}

                def mk_gather(b):
                    return lambda: state.__setitem__(b, gather_sample(b))

                def mk_corr(b):
                    return lambda: corr_sample(b, state[b][0], state[b][1])

                def mk_gates(b):
                    return lambda: gates_sample(b, state[b][0], state[b][2])

                b0, b1 = 2 * pr, 2 * pr + 1
                return [mk_gather(b0), mk_gather(b1),
                        mk_corr(b0), mk_corr(b1),
                        mk_gates(b0), mk_gates(b1)]

            # ---------------- scan phase ----------------
            Mts = {}
            B86 = CH + 1  # 86 columns per c-block (reset col + 85 updates)
            Sfull = [pp.tile([128, SC * B86], f32, tag=f"Sf{p}", name=f"Sf{p}")
                     for p in range(2)]

            preps = {}

            def prep_chunk(pr, ch):
                s0 = CH * ch
                # wb stored s-major (contiguous replicate-DMA); compute
                # ops read it through a permuted (c-outer) view
                wb = cp2.tile([128, C * CH], f32, tag="wb")
                wb3 = wb[:].rearrange("p (s c) -> p s c", c=C) \
                           .rearrange("p s c -> p c s")
                for bb in range(2):
                    srcb = w_dram[2 * pr + bb, s0:s0 + CH, :]
                    srcb = srcb[None, :, :].to_broadcast([D, CH, C])
                    nc.sync.dma_start(wb[D * bb:D * bb + D, 0:2000],
                                      srcb[:, 0:40, :])
                    nc.sync.dma_start(wb[D * bb:D * bb + D, 2000:C * CH],
                                      srcb[:, 40:CH, :])

                u = cp2.tile([128, C * (CH + 1)], f16, tag="u")
                v = cp2.tile([128, C * (CH + 1)], f16, tag="v")
                u3 = u[:].rearrange("p (c s) -> p c s", s=CH + 1)
                v3 = v[:].rearrange("p (c s) -> p c s", s=CH + 1)

                Eb = E[pr][:, s0:s0 + CH][:, None, :].to_broadcast([128, C, CH])
                Ab = A[pr][:, s0:s0 + CH][:, None, :].to_broadcast([128, C, CH])
                nc.vector.tensor_tensor(u3[:, :, 1:], wb3, Eb, op=OP.mult)
                nc.scalar.activation(u3[:, :, 1:], u3[:, :, 1:], AF.Copy,
                                     bias=1.0, scale=-1.0)
                nc.gpsimd.memset(u3[:, :, 0:1], 0.0)
                nc.gpsimd.tensor_tensor(v3[:, :, 1:], wb3, Ab, op=OP.mult)
                preps[(pr, ch)] = (wb3, u, v, v3)

            def exec_chunk(pr, ch):
                wb3, u, v, v3 = preps.pop((pr, ch))
                Mt = cp1.tile([128, C * (CH + 1)], f32, tag="Mt")
                Mt3 = Mt[:].rearrange("p (c s) -> p c s", s=CH + 1)
                # reset column: carry the state across chunks.  STT instead of
                # tensor_copy: strided-src tensor_copy hits a pathological DVE
                # mode (~11us for 50 elements).
                if ch == 0:
                    carry_src = M0T[:][:, :, None]
                else:
                    carry_src = Mts[pr][:, :, CH:CH + 1]
                nc.vector.scalar_tensor_tensor(
                    out=v3[:, :, 0:1], in0=carry_src, scalar=1.0,
                    op0=OP.mult, in1=ones50[:][:, :, None], op1=OP.mult)

                nc.vector.tensor_tensor_scan(
                    Mt[:], u[:], v[:], 0.0, op0=OP.mult, op1=OP.add)
                Mts[pr] = Mt3

                # q_read: small per-step STTs against the step-start state
                scr2 = cp2.tile([128, C], f32, tag="scr2")
                for tl in range(TC):
                    t = TC * ch + tl
                    sl = 17 * tl
                    nc.vector.scalar_tensor_tensor(
                        out=scr2[:][:, :, None],
                        in0=wb3[:, :, sl + 16:sl + 17],
                        scalar=1.0, op0=OP.mult, in1=Mt3[:, :, sl:sl + 1],
                        op1=OP.mult, accum_out=qr[pr][:, t:t + 1])

                # S-tree: SS[p, s] = sum_c Mt[p, c, s] (for telescoped l_read)
                # 50 = 25+25 -> 12+12 (+1 leftover) -> 6 -> 3 -> 1+1+1
                t25 = cp2.tile([128, 25 * B86], f32, tag="t25")
                nc.gpsimd.tensor_tensor(t25[:], Mt[:, 0:25 * B86],
                                        Mt[:, 25 * B86:50 * B86], op=OP.add)
                t12 = cp1.tile([128, 12 * B86], f32, tag="t12")
                nc.vector.tensor_tensor(t12[:], t25[:, 0:12 * B86],
                                        t25[:, 12 * B86:24 * B86], op=OP.add)
                t6 = cp1.tile([128, 6 * B86], f32, tag="t6")
                nc.vector.tensor_tensor(t6[:], t12[:, 0:6 * B86],
                                        t12[:, 6 * B86:12 * B86], op=OP.add)
                t3 = cp1.tile([128, 3 * B86], f32, tag="t3")
                nc.vector.tensor_tensor(t3[:], t6[:, 0:3 * B86],
                                        t6[:, 3 * B86:6 * B86], op=OP.add)
                t1 = cp1.tile([128, B86], f32, tag="t1")
                nc.vector.tensor_tensor(t1[:], t3[:, 0:B86],
                                        t3[:, B86:2 * B86], op=OP.add)
                nc.vector.tensor_tensor(t1[:], t1[:],
                                        t3[:, 2 * B86:3 * B86], op=OP.add)
                nc.vector.tensor_tensor(
                    Sfull[pr][:, ch * B86:(ch + 1) * B86], t1[:],
                    t25[:, 24 * B86:25 * B86], op=OP.add)

            def fixup_pair(pr):
                """Telescoped l_read: for each write, read = (S_before -
                S_after + a) / e; sum the 16 lecture reads per step."""
                Sf3 = Sfull[pr][:].rearrange("p (h s) -> p h s", s=B86)
                ds = cp1.tile([128, NS], f32, tag="ds")
                ds3 = ds[:].rearrange("p (h s) -> p h s", s=CH)
                nc.vector.tensor_tensor(ds3, Sf3[:, :, 0:CH],
                                        Sf3[:, :, 1:B86], op=OP.subtract)
                # r = 1/E by two Newton steps from r0=2 (E = sigmoid, ~0.5)
                r = cp1.tile([128, NS], f32, tag="rr")
                tn = cp1.tile([128, NS], f32, tag="tn")
                nc.scalar.activation(r[:], E[pr][:, :], AF.Copy,
                                     scale=-4.0, bias=4.0)
                nc.vector.tensor_tensor(tn[:], E[pr][:, :], r[:], op=OP.mult)
                nc.scalar.activation(tn[:], tn[:], AF.Copy,
                                     scale=-1.0, bias=2.0)
                nc.vector.tensor_tensor(r[:], r[:], tn[:], op=OP.mult)
                nc.vector.tensor_tensor(tn[:], E[pr][:, :], r[:], op=OP.mult)
                nc.scalar.activation(tn[:], tn[:], AF.Copy,
                                     scale=-1.0, bias=2.0)
                nc.vector.tensor_tensor(r[:], r[:], tn[:], op=OP.mult)
                # zero the qa positions so the 17-wide reduce skips them
                r3 = r[:].rearrange("p (t k) -> p t k", k=17)
                nc.gpsimd.memset(r3[:, :, 16:17], 0.0)
                nc.vector.tensor_tensor(ds[:], ds[:], A[pr][:, :], op=OP.add)
                nc.vector.tensor_tensor(ds[:], ds[:], r[:], op=OP.mult)
                nc.vector.tensor_reduce(
                    lr[pr][:, :], ds[:].rearrange("p (t k) -> p t k", k=17),
                    axis=AX.X, op=OP.add)

            # ---------------- readout: mastery -> LN -> MLP ----------------
            msT_lo = pp.tile([128, BL * S], f32, tag="msTlo")
            msT_hi = pp.tile([128, BL * S], f32, tag="msThi")

            def readout_pair(pr):
                ms = pp.tile([S, 2 * H4], f32, tag=f"ms{pr}")
                for which, tsrc in ((0, qr[pr]), (2, lr[pr])):
                    pst = psp.tile([S, 128], f32, space="PSUM", tag="tp")
                    nc.tensor.transpose(out=pst[:], in_=tsrc[:], identity=ident[:])
                    for bh in range(2):
                        nc.vector.tensor_copy(
                            ms[:, bh * H4 + which * D:bh * H4 + (which + 1) * D],
                            pst[:, bh * D:(bh + 1) * D])
                for bh in range(2):
                    b = 2 * pr + bh
                    nc.vector.tensor_copy(ms[:, bh * H4 + D:bh * H4 + 2 * D],
                                          q_raw[b][0:S, :])
                    nc.vector.tensor_copy(ms[:, bh * H4 + 3 * D:bh * H4 + 4 * D],
                                          le_raw[b][0:S, :])
                ms3 = ms[:].rearrange("p (b f) -> p b f", f=H4)
                mean = pp.tile([S, 2], f32, tag=f"mean{pr}")
                nc.vector.tensor_reduce(mean[:], ms3, axis=AX.X, op=OP.add)
                nc.vector.tensor_scalar_mul(mean[:], mean[:], 1.0 / H4)
                mb = mean[:][:, :, None].to_broadcast([S, 2, H4])
                nc.vector.tensor_tensor(ms3, ms3, mb, op=OP.subtract)
                sq = pp.tile([S, 2 * H4], f32, tag=f"sq{pr}")
                nc.scalar.activation(sq[:], ms[:], AF.Square)
                var = pp.tile([S, 2], f32, tag=f"var{pr}")
                nc.vector.tensor_reduce(
                    var[:], sq[:].rearrange("p (b f) -> p b f", f=H4),
                    axis=AX.X, op=OP.add)
                nc.vector.tensor_scalar(var[:], var[:], 1.0 / H4, EPS,
                                        op0=OP.mult, op1=OP.add)
                sd = pp.tile([S, 2], f32, tag=f"sd{pr}")
                nc.scalar.activation(sd[:], var[:], AF.Sqrt)
                rsd = pp.tile([S, 2], f32, tag=f"rsd{pr}")
                nc.vector.reciprocal(rsd[:], sd[:])
                nc.vector.tensor_tensor(
                    ms3, ms3, rsd[:][:, :, None].to_broadcast([S, 2, H4]),
                    op=OP.mult)
                gmb = gb_rep[:, 0:H4][:, None, :].to_broadcast([S, 2, H4])
                btb = gb_rep[:, H4:2 * H4][:, None, :].to_broadcast([S, 2, H4])
                nc.vector.tensor_tensor(ms3, ms3, gmb, op=OP.mult)
                nc.vector.tensor_tensor(ms3, ms3, btb, op=OP.add)
                for bh in range(2):
                    b = 2 * pr + bh
                    for fh, dstT in ((0, msT_lo), (1, msT_hi)):
                        pst = psp.tile([128, S], f32, space="PSUM", tag="tp")
                        nc.tensor.transpose(
                            out=pst[:],
                            in_=ms[:, bh * H4 + fh * 128:bh * H4 + (fh + 1) * 128],
                            identity=ident[0:S, 0:S])
                        nc.vector.tensor_copy(dstT[:, b * S:(b + 1) * S], pst[:])

            def mlp_pair(rc):
                rows = 2 * S  # 100 rows: (b within pair, t)
                csl = slice(rc * rows, (rc + 1) * rows)
                ph = psp.tile([rows, H4], f32, space="PSUM", tag="mlp")
                nc.tensor.matmul(ph[:], lhsT=msT_lo[:, csl], rhs=W01[:, 0, :],
                                 start=True, stop=False)
                nc.tensor.matmul(ph[:], lhsT=msT_hi[:, csl], rhs=W01[:, 1, :],
                                 start=False, stop=True)
                h1 = pp.tile([rows, H4], f32, tag=f"h1_{rc}")
                nc.vector.tensor_tensor(h1[:], ph[:], bias_rep[0:rows, 0:H4],
                                        op=OP.add)
                nc.scalar.activation(h1[:], h1[:], AF.Relu)
                h1T = [pp.tile([128, rows], f32, tag=f"h1T{fh}_{rc}", name=f"h1T{fh}_{rc}")
                       for fh in range(2)]
                for fh in range(2):
                    pst = psp.tile([128, rows], f32, space="PSUM", tag="tp")
                    nc.tensor.transpose(out=pst[:],
                                        in_=h1[:, fh * 128:(fh + 1) * 128],
                                        identity=ident[0:rows, 0:rows])
                    nc.vector.tensor_copy(h1T[fh][:], pst[:])
                ph2 = psp.tile([rows, H4], f32, space="PSUM", tag="mlp")
                nc.tensor.matmul(ph2[:], lhsT=h1T[0][:], rhs=W01[:, 2, :],
                                 start=True, stop=False)
                nc.tensor.matmul(ph2[:], lhsT=h1T[1][:], rhs=W01[:, 3, :],
                                 start=False, stop=True)
                h2 = pp.tile([rows, H4], f32, tag=f"h2_{rc}")
                nc.vector.tensor_tensor(h2[:], ph2[:],
                                        bias_rep[0:rows, H4:2 * H4], op=OP.add)
                scr4 = pp.tile([rows, H4], f32, tag=f"scr4_{rc}")
                logit = pp.tile([rows, 1], f32, tag=f"logit{rc}")
                nc.vector.scalar_tensor_tensor(
                    out=scr4[:], in0=h2[:], scalar=1.0, op0=OP.mult,
                    in1=Wout_rep[0:rows, :], op1=OP.mult,
                    accum_out=logit[:, 0:1])
                psig = pp.tile([rows, 1], f32, tag=f"psig{rc}")
                nc.scalar.activation(psig[:], logit[:], AF.Sigmoid,
                                     bias=bout_rep[0:rows, 0:1], scale=1.0)
                nc.sync.dma_start(
                    preds[2 * rc:2 * rc + 2, :].rearrange("b t -> (b t)")[:, None],
                    psig[:, 0:1])

            # ---------------- pipelined emission ----------------
            # pair-0 gather
            for piece in gather_pair(0):
                piece()
            # pair-0 scan, pair-1 gather interleaved; prep runs one chunk
            # ahead of exec so the scan never waits on its own u/v chain
            p1_pieces = gather_pair(1)
            slots = [(0, ch) for ch in range(SC)] + [(1, ch) for ch in range(SC)]
            prep_chunk(*slots[0])
            for i, (pr, ch) in enumerate(slots):
                if i + 1 < len(slots):
                    prep_chunk(*slots[i + 1])
                exec_chunk(pr, ch)
                if pr == 0 and ch < len(p1_pieces):
                    p1_pieces[ch]()
                if pr == 0 and ch == SC - 1:
                    fixup_pair(0)
                if pr == 1 and ch == 0:
                    readout_pair(0)
                elif pr == 1 and ch == 1:
                    mlp_pair(0)
            fixup_pair(1)
            readout_pair(1)
            mlp_pair(1)

    nc.compile()
    return nc


def _host_prepare(inputs):
    q_data = np.asarray(inputs["q_data"]).astype(np.int32)
    qa_data = np.asarray(inputs["qa_data"]).astype(np.int32)
    l_data = np.asarray(inputs["l_data"]).astype(np.int32)
    f = lambda k: np.ascontiguousarray(np.asarray(inputs[k]), dtype=np.float32)
    q_embed, qa_embed = f("q_embed"), f("qa_embed")
    key, M0 = f("key_matrix"), f("M0")
    W_ea = np.concatenate([f("W_e"), f("W_a")], axis=1)
    W_ea = np.concatenate(
        [W_ea, np.concatenate([f("b_e"), f("b_a")])[None, :]], axis=0)
    biases = np.concatenate([f("b0"), f("b1"), f("b_e"), f("b_a")])
    gamma_beta = np.concatenate([f("ln_gamma"), f("ln_beta")])
    W0, W1 = f("W0"), f("W1")
    Wout = f("W_out").reshape(-1)
    b_out = f("b_out").reshape(-1)

    in_maps = []
    for core in range(NCORES):
        bs = slice(core * BL, (core + 1) * BL)
        ql = np.zeros((BL, LPAD), np.int32)
        ql[:, :LROWS] = l_data[bs].reshape(BL, LROWS)
        qi = np.zeros((BL, QPAD), np.int32)
        qi[:, :S] = q_data[bs]
        qai = np.zeros((BL, QPAD), np.int32)
        qai[:, :S] = qa_data[bs]
        lei = np.zeros((BL, QPAD), np.int32)
        lei[:, :S] = l_data[bs][:, :, L - 1]
        in_maps.append(dict(
            ql_idx=np.ascontiguousarray(ql.reshape(-1)),
            q_idx=np.ascontiguousarray(qi.reshape(-1)),
            qa_idx=np.ascontiguousarray(qai.reshape(-1)),
            le_idx=np.ascontiguousarray(lei.reshape(-1)),
            q_embed=q_embed, qa_embed=qa_embed, key=key, M0=M0,
            W_ea=W_ea, W0=W0, W1=W1, Wout=Wout, biases=biases,
            gamma_beta=gamma_beta, b_out=b_out,
        ))
    return in_maps


def kernel(**inputs):
    global _BUILT
    if _BUILT is None:
        _BUILT = _build()
    nc = _BUILT
    from concourse import bass_utils
    in_maps = _host_prepare(inputs)
    res = bass_utils.run_bass_kernel_spmd(
        nc, in_maps, core_ids=list(range(NCORES)),
        trace=bool(int(os.environ.get("KERNEL_TRACE", "0"))))
    out = np.concatenate([r["preds"] for r in res.results], axis=0)
    kernel.last_results = res
    return out
